# revision 12
# baseline (speedup 1.0000x reference)
"""Trainium2 Bass kernel for a pre-LN transformer block (nn_BaseBlock).

Reference computation (per batch b, fp32):
    h   = LN1(x); k,q,v = h@Wk+bk, h@Wq+bq, h@Wv+bv
    sim = (k @ q^T)/sqrt(E)  (causal tril mask), att = softmax(sim) @ v
    x2  = x + att
    h2  = LN2(x2)
    f   = gelu(gelu(gelu(h2@W1+b1)@W2a+b2a)@W2b+b2b)@W3 + b3
    out = x2 + f

Sharding over 8 cores: core c handles batch b=c//2, row half r=c%2
(rows [r*1024:(r+1)*1024) of that batch).  Every core computes full-context
q/v for its batch (cheap duplication) so a single SPMD program runs on all
cores; causality and row position enter only through a per-core mask input.

On-chip layout: activations are kept feature-major ("T" suffix = transposed,
[feature, token]) for matmul chaining; LayerNorm/softmax statistics are
computed token-major in fp32.  Matmul inputs are bf16 (fp32 PSUM accum);
residual stream stays fp32.
"""

import time

import numpy as np
import ml_dtypes

import concourse.bass as bass
import concourse.mybir as mybir
from concourse import bacc
import concourse.tile as tile
from concourse.bass_utils import run_bass_kernel_spmd
from concourse.masks import make_identity

F32 = mybir.dt.float32
BF16 = mybir.dt.bfloat16
AF = mybir.ActivationFunctionType
ALU = mybir.AluOpType
AX = mybir.AxisListType

EPS = 1e-5
N_CORES = 8


class Cfg:
    def __init__(self, E=1024, H=4096, T=2048, R=1024):
        self.E, self.H, self.T, self.R = E, H, T, R
        self.ET, self.HT, self.CT, self.RT = E // 128, H // 128, T // 128, R // 128
        self.scale = 1.0 / np.sqrt(E)


def _blocks(total, bs=512):
    return [(o, min(bs, total - o)) for o in range(0, total, bs)]


def build_program(cfg: Cfg):
    """Build the SPMD Bass program (one core's view)."""
    E, H, T, R = cfg.E, cfg.H, cfg.T, cfg.R
    ET, HT, CT, RT = cfg.ET, cfg.HT, cfg.CT, cfg.RT
    EB = _blocks(E)
    TB = _blocks(T)
    RB = _blocks(R)

    nc = bacc.Bacc("TRN2", target_bir_lowering=False, debug=False,
                   num_devices=N_CORES)

    # ---- DRAM I/O ----
    x_b = nc.dram_tensor("x_b", [T, E], F32, kind="ExternalInput")
    x_own = nc.dram_tensor("x_own", [R, E], F32, kind="ExternalInput")
    mask = nc.dram_tensor("mask", [R, T], F32, kind="ExternalInput")
    ln1_w = nc.dram_tensor("ln1_w", [E], F32, kind="ExternalInput")
    ln1_b = nc.dram_tensor("ln1_b", [E], F32, kind="ExternalInput")
    ln2_w = nc.dram_tensor("ln2_w", [E], F32, kind="ExternalInput")
    ln2_b = nc.dram_tensor("ln2_b", [E], F32, kind="ExternalInput")
    wqt = nc.dram_tensor("wqt", [ET, 128, ET, 128], BF16, kind="ExternalInput")
    wkt = nc.dram_tensor("wkt", [ET, 128, ET, 128], BF16, kind="ExternalInput")
    wv = nc.dram_tensor("wv", [E, E], BF16, kind="ExternalInput")
    bq = nc.dram_tensor("bq", [E], F32, kind="ExternalInput")
    bk = nc.dram_tensor("bk", [E], F32, kind="ExternalInput")
    bv = nc.dram_tensor("bv", [E], F32, kind="ExternalInput")
    w1t = nc.dram_tensor("w1t", [HT, 128, ET, 128], BF16, kind="ExternalInput")
    w2at = nc.dram_tensor("w2at", [HT, 128, HT, 128], BF16, kind="ExternalInput")
    w2bt = nc.dram_tensor("w2bt", [HT, 128, HT, 128], BF16, kind="ExternalInput")
    b1 = nc.dram_tensor("b1", [H], F32, kind="ExternalInput")
    b2a = nc.dram_tensor("b2a", [H], F32, kind="ExternalInput")
    b2b = nc.dram_tensor("b2b", [H], F32, kind="ExternalInput")
    w3t = nc.dram_tensor("w3t", [len(EB), HT, 128, EB[0][1]], BF16,
                         kind="ExternalInput")
    b3 = nc.dram_tensor("b3", [E], F32, kind="ExternalInput")
    out = nc.dram_tensor("out", [R, E], F32, kind="ExternalOutput")

    with tile.TileContext(nc) as tc:
        _emit(tc, cfg, locals())
    nc.compile()
    return nc


def _ln_tile(nc, pools, x_ap, w_bc, b_bc, out_bf, eps_t, E):
    """LayerNorm one [128, E] token tile: out_bf (bf16) = LN(x)*w + b.

    x_ap may be SBUF or a tile just DMA'd in; it is modified in place.
    """
    nsub = E // 512 if E >= 512 else 1
    sub = E // nsub
    stats = pools["ln_stats"].tile([128, nsub, 6], F32, tag="ln_stats")
    xr = x_ap.rearrange("p (n s) -> p n s", n=nsub)
    for i in range(nsub):
        nc.vector.bn_stats(out=stats[:, i, :], in_=xr[:, i, :])
    mv = pools["ln_stats"].tile([128, 2], F32, tag="ln_mv")
    nc.vector.bn_aggr(out=mv[:], in_=stats[:])
    sd = pools["ln_stats"].tile([128, 1], F32, tag="ln_sd")
    nc.scalar.activation(out=sd[:], in_=mv[:, 1:2], func=AF.Sqrt,
                         bias=eps_t[:], scale=1.0)
    rinv = pools["ln_stats"].tile([128, 1], F32, tag="ln_rinv")
    nc.vector.reciprocal(out=rinv[:], in_=sd[:])
    nc.vector.tensor_scalar(out=x_ap, in0=x_ap, scalar1=mv[:, 0:1],
                            scalar2=rinv[:], op0=ALU.subtract, op1=ALU.mult)
    nc.vector.tensor_tensor(out=x_ap, in0=x_ap, in1=w_bc, op=ALU.mult)
    nc.vector.tensor_tensor(out=out_bf, in0=x_ap, in1=b_bc, op=ALU.add)


def _emit(tc, cfg, d):
    nc = tc.nc
    E, H, T, R = cfg.E, cfg.H, cfg.T, cfg.R
    ET, HT, CT, RT = cfg.ET, cfg.HT, cfg.CT, cfg.RT
    EB, TB, RB = _blocks(E), _blocks(T), _blocks(R)
    x_b, x_own, mask, out = d["x_b"], d["x_own"], d["mask"], d["out"]

    import contextlib
    ctx = contextlib.ExitStack()
    with ctx:
        # ---------- constant / persistent pools ----------
        consts = ctx.enter_context(tc.tile_pool(name="consts", bufs=1))
        mm_ps = ctx.enter_context(tc.tile_pool(name="mm_ps", bufs=4, space="PSUM"))
        tr_ps = ctx.enter_context(tc.tile_pool(name="tr_ps", bufs=2, space="PSUM"))
        pools = {"ln_stats": ctx.enter_context(tc.tile_pool(name="ln_stats", bufs=3))}

        eps_t = consts.tile([128, 1], F32)
        nc.vector.memset(eps_t[:], EPS)
        ident = consts.tile([128, 128], BF16)
        make_identity(nc, ident[:])

        def bcast(name, dr, dtype=BF16, width=None):
            w = width or dr.shape[0]
            t = consts.tile([128, w], dtype, tag=name)
            src = dr.ap()
            src_b = bass.AP(tensor=src.tensor, offset=src.offset,
                            ap=[[0, 128]] + list(src.ap))
            eng = nc.gpsimd if dtype != dr.dtype else nc.sync
            eng.dma_start(out=t[:], in_=src_b)
            return t

        def cols(name, dr, nt):
            t = consts.tile([128, nt], F32, tag=name)
            nc.sync.dma_start(out=t[:], in_=dr.ap().rearrange("(t p) -> p t", p=128))
            return t

        ln1w_bc = bcast("ln1w", d["ln1_w"])
        ln1b_bc = bcast("ln1b", d["ln1_b"])
        ln2w_bc = bcast("ln2w", d["ln2_w"])
        ln2b_bc = bcast("ln2b", d["ln2_b"])
        bv_bc = bcast("bv", d["bv"])
        b3_bc = bcast("b3", d["b3"], dtype=F32)
        bq_c = cols("bq", d["bq"], ET)
        bk_c = cols("bk", d["bk"], ET)
        b1_c = cols("b1", d["b1"], HT)
        b2a_c = cols("b2a", d["b2a"], HT)
        b2b_c = cols("b2b", d["b2b"], HT)

        x2_pool = ctx.enter_context(tc.tile_pool(name="x2", bufs=1))
        x2 = x2_pool.tile([128, RT, E], F32)  # residual stream (own rows), fp32

        # ================= attention block =================
        with tc.tile_pool(name="attn_big", bufs=1) as abig:
            qT = abig.tile([128, ET, T], BF16, tag="qT")
            kT = abig.tile([128, ET, R], BF16, tag="kT")
            vtm = abig.tile([128, CT, E], BF16, tag="vtm")  # token-major v

            with tc.tile_pool(name="hT_pool", bufs=1) as hp:
                hT = hp.tile([128, ET, T], BF16, tag="hT")
                hTo = hp.tile([128, ET, R], BF16, tag="hTo")

                # ---- phase 1: LN1 + transpose to feature-major ----
                with tc.tile_pool(name="ln_work", bufs=3) as lw, \
                     tc.tile_pool(name="ln_out", bufs=3) as lo:
                    def ln_transpose(src, n_tiles, dstT):
                        for t in range(n_tiles):
                            xt = lw.tile([128, E], F32, tag="xt")
                            nc.sync.dma_start(out=xt[:], in_=src[t * 128:(t + 1) * 128, :])
                            hbf = lo.tile([128, E], BF16, tag="hbf")
                            _ln_tile(nc, pools, xt[:], ln1w_bc[:], ln1b_bc[:],
                                     hbf[:], eps_t, E)
                            for et in range(ET):
                                tp = tr_ps.tile([128, 128], BF16, tag="tr")
                                nc.tensor.transpose(tp[:], hbf[:, et * 128:(et + 1) * 128],
                                                    ident[:])
                                nc.vector.tensor_copy(
                                    out=dstT[:, et, t * 128:(t + 1) * 128], in_=tp[:])
                    ln_transpose(x_b.ap(), CT, hT)
                    ln_transpose(x_own.ap(), RT, hTo)

                # ---- phase 2a: q (full ctx) and k (own rows), feature-major ----
                with tc.tile_pool(name="wqk_stream", bufs=3) as ws:
                    for mt in range(ET):
                        wq_mt = ws.tile([128, ET, 128], BF16, tag="wq_mt")
                        nc.sync.dma_start(out=wq_mt[:], in_=d["wqt"].ap()[mt])
                        for jo, jn in TB:
                            ps = mm_ps.tile([128, 512], F32, tag="mm")
                            for kt in range(ET):
                                nc.tensor.matmul(ps[:, :jn], wq_mt[:, kt, :],
                                                 hT[:, kt, jo:jo + jn],
                                                 start=(kt == 0), stop=(kt == ET - 1))
                            nc.scalar.activation(
                                out=qT[:, mt, jo:jo + jn], in_=ps[:, :jn],
                                func=AF.Identity, bias=bq_c[:, mt:mt + 1], scale=1.0)
                        wk_mt = ws.tile([128, ET, 128], BF16, tag="wk_mt")
                        nc.sync.dma_start(out=wk_mt[:], in_=d["wkt"].ap()[mt])
                        for ro, rn in RB:
                            ps = mm_ps.tile([128, 512], F32, tag="mm")
                            for kt in range(ET):
                                nc.tensor.matmul(ps[:, :rn], wk_mt[:, kt, :],
                                                 hTo[:, kt, ro:ro + rn],
                                                 start=(kt == 0), stop=(kt == ET - 1))
                            nc.scalar.activation(
                                out=kT[:, mt, ro:ro + rn], in_=ps[:, :rn],
                                func=AF.Identity, bias=bk_c[:, mt:mt + 1], scale=1.0)

                # ---- phase 2b: v (full ctx), token-major ----
                with tc.tile_pool(name="wv_pool", bufs=1) as wvp:
                    wv_sb = wvp.tile([128, ET, E], BF16)
                    nc.sync.dma_start(out=wv_sb[:],
                                      in_=d["wv"].ap().rearrange("(kt p) e -> p kt e", p=128))
                    for tt in range(CT):
                        for eo, en in EB:
                            ps = mm_ps.tile([128, 512], F32, tag="mm")
                            for kt in range(ET):
                                nc.tensor.matmul(ps[:, :en], hT[:, kt, tt * 128:(tt + 1) * 128],
                                                 wv_sb[:, kt, eo:eo + en],
                                                 start=(kt == 0), stop=(kt == ET - 1))
                            nc.vector.tensor_tensor(
                                out=vtm[:, tt, eo:eo + en], in0=ps[:, :en],
                                in1=bv_bc[:, eo:eo + en], op=ALU.add)

            # ---- phase 3: attention rows (own i-tiles) ----
            with tc.tile_pool(name="at_mask", bufs=2) as mkp, \
                 tc.tile_pool(name="at_sim", bufs=2) as smp, \
                 tc.tile_pool(name="at_p", bufs=2) as pp, \
                 tc.tile_pool(name="at_misc", bufs=3) as msc:
                for it in range(RT):
                    mk = mkp.tile([128, T], F32, tag="mk")
                    nc.sync.dma_start(out=mk[:], in_=mask.ap()[it * 128:(it + 1) * 128, :])
                    sim = smp.tile([128, T], F32, tag="sim")
                    for jo, jn in TB:
                        ps = mm_ps.tile([128, 512], F32, tag="mm")
                        for et in range(ET):
                            nc.tensor.matmul(ps[:, :jn], kT[:, et, it * 128:(it + 1) * 128],
                                             qT[:, et, jo:jo + jn],
                                             start=(et == 0), stop=(et == ET - 1))
                        nc.vector.tensor_tensor(out=sim[:, jo:jo + jn], in0=ps[:, :jn],
                                                in1=mk[:, jo:jo + jn], op=ALU.add)
                    mneg = msc.tile([128, 1], F32, tag="mneg")
                    nc.vector.tensor_reduce(out=mneg[:], in_=sim[:], axis=AX.X,
                                            op=ALU.max, negate=True)
                    msc_t = msc.tile([128, 1], F32, tag="msc")
                    nc.scalar.mul(out=msc_t[:], in_=mneg[:], mul=float(cfg.scale))
                    pbf = pp.tile([128, T], BF16, tag="pbf")
                    lrow = msc.tile([128, 1], F32, tag="lrow")
                    nc.scalar.activation(out=pbf[:], in_=sim[:], func=AF.Exp,
                                         scale=float(cfg.scale), bias=msc_t[:],
                                         accum_out=lrow[:])
                    linv = msc.tile([128, 1], F32, tag="linv")
                    nc.vector.reciprocal(out=linv[:], in_=lrow[:])
                    pT = pp.tile([128, T], BF16, tag="pT")
                    for jt in range(CT):
                        tp = tr_ps.tile([128, 128], BF16, tag="tr")
                        nc.tensor.transpose(tp[:], pbf[:, jt * 128:(jt + 1) * 128], ident[:])
                        nc.vector.tensor_copy(out=pT[:, jt * 128:(jt + 1) * 128], in_=tp[:])
                    xo = msc.tile([128, E], F32, tag="xo")
                    nc.sync.dma_start(out=xo[:], in_=x_own.ap()[it * 128:(it + 1) * 128, :])
                    for eo, en in EB:
                        ps = mm_ps.tile([128, 512], F32, tag="mm")
                        for jt in range(CT):
                            nc.tensor.matmul(ps[:, :en], pT[:, jt * 128:(jt + 1) * 128],
                                             vtm[:, jt, eo:eo + en],
                                             start=(jt == 0), stop=(jt == CT - 1))
                        nc.vector.scalar_tensor_tensor(
                            out=x2[:, it, eo:eo + en], in0=ps[:, :en], scalar=linv[:],
                            in1=xo[:, eo:eo + en], op0=ALU.mult, op1=ALU.add)

        # ================= MLP block =================
        with tc.tile_pool(name="mlp_big", bufs=1) as mbig:
            h2T = mbig.tile([128, ET, R], BF16, tag="h2T")

            # ---- phase 4: LN2 + transpose; then fold b3 into x2 ----
            with tc.tile_pool(name="ln2_work", bufs=2) as l2w, \
                 tc.tile_pool(name="ln2_out", bufs=2) as l2o:
                for rt in range(RT):
                    ht = l2w.tile([128, E], F32, tag="h2tmp")
                    nc.vector.tensor_copy(out=ht[:], in_=x2[:, rt, :])
                    h2bf = l2o.tile([128, E], BF16, tag="h2bf")
                    _ln_tile(nc, pools, ht[:], ln2w_bc[:], ln2b_bc[:], h2bf[:],
                             eps_t, E)
                    for et in range(ET):
                        tp = tr_ps.tile([128, 128], BF16, tag="tr")
                        nc.tensor.transpose(tp[:], h2bf[:, et * 128:(et + 1) * 128], ident[:])
                        nc.vector.tensor_copy(out=h2T[:, et, rt * 128:(rt + 1) * 128],
                                              in_=tp[:])
                    nc.vector.tensor_tensor(out=x2[:, rt, :], in0=x2[:, rt, :],
                                            in1=b3_bc[:], op=ALU.add)

            with tc.tile_pool(name="gx", bufs=1) as gxp:
                g1T = gxp.tile([128, HT, R], BF16, tag="gx")
                # ---- g1 = gelu(h2 @ W1 + b1), feature-major ----
                with tc.tile_pool(name="w1_stream", bufs=3) as w1s:
                    for mt in range(HT):
                        w1_mt = w1s.tile([128, ET, 128], BF16, tag="w1_mt")
                        nc.sync.dma_start(out=w1_mt[:], in_=d["w1t"].ap()[mt])
                        for ro, rn in RB:
                            ps = mm_ps.tile([128, 512], F32, tag="mm")
                            for kt in range(ET):
                                nc.tensor.matmul(ps[:, :rn], w1_mt[:, kt, :],
                                                 h2T[:, kt, ro:ro + rn],
                                                 start=(kt == 0), stop=(kt == ET - 1))
                            nc.scalar.activation(out=g1T[:, mt, ro:ro + rn], in_=ps[:, :rn],
                                                 func=AF.Gelu, bias=b1_c[:, mt:mt + 1],
                                                 scale=1.0)

                # ---- g2 = gelu(g1 @ W2a + b2a); g3 = gelu(g2 @ W2b + b2b) ----
                # g3T reuses g1T's slot (same pool+tag); g2 pool closes before
                # the f-phase pools open so its range can be reused.
                with tc.tile_pool(name="g2", bufs=1) as g2p:
                    g2T = g2p.tile([128, HT, R], BF16, tag="g2")
                    with tc.tile_pool(name="w2a_stream", bufs=2) as w2s:
                        for mt in range(HT):
                            w2_mt = w2s.tile([128, HT, 128], BF16, tag="w2_mt")
                            nc.sync.dma_start(out=w2_mt[:], in_=d["w2at"].ap()[mt])
                            for ro, rn in RB:
                                ps = mm_ps.tile([128, 512], F32, tag="mm")
                                for kt in range(HT):
                                    nc.tensor.matmul(ps[:, :rn], w2_mt[:, kt, :],
                                                     g1T[:, kt, ro:ro + rn],
                                                     start=(kt == 0), stop=(kt == HT - 1))
                                nc.scalar.activation(out=g2T[:, mt, ro:ro + rn],
                                                     in_=ps[:, :rn], func=AF.Gelu,
                                                     bias=b2a_c[:, mt:mt + 1], scale=1.0)

                    g3T = gxp.tile([128, HT, R], BF16, tag="gx")
                    with tc.tile_pool(name="w2b_stream", bufs=2) as w2s2:
                        for mt in range(HT):
                            w2_mt = w2s2.tile([128, HT, 128], BF16, tag="w2b_mt")
                            nc.sync.dma_start(out=w2_mt[:], in_=d["w2bt"].ap()[mt])
                            for ro, rn in RB:
                                ps = mm_ps.tile([128, 512], F32, tag="mm")
                                for kt in range(HT):
                                    nc.tensor.matmul(ps[:, :rn], w2_mt[:, kt, :],
                                                     g2T[:, kt, ro:ro + rn],
                                                     start=(kt == 0), stop=(kt == HT - 1))
                                nc.scalar.activation(out=g3T[:, mt, ro:ro + rn],
                                                     in_=ps[:, :rn], func=AF.Gelu,
                                                     bias=b2b_c[:, mt:mt + 1], scale=1.0)

                # ---- f = g3 @ W3 (+b3 already in x2); out = x2 + f ----
                with tc.tile_pool(name="w3_pool", bufs=1) as w3p, \
                     tc.tile_pool(name="out_pool", bufs=3) as op:
                    for ebi, (eo, en) in enumerate(EB):
                        w3_sb = w3p.tile([128, HT, EB[0][1]], BF16, tag="w3_sb")
                        nc.sync.dma_start(out=w3_sb[:],
                                          in_=d["w3t"].ap()[ebi].rearrange("kt p e -> p kt e"))
                        for tt in range(RT):
                            ps = mm_ps.tile([128, 512], F32, tag="mm")
                            for kt in range(HT):
                                nc.tensor.matmul(ps[:, :en],
                                                 g3T[:, kt, tt * 128:(tt + 1) * 128],
                                                 w3_sb[:, kt, :en],
                                                 start=(kt == 0), stop=(kt == HT - 1))
                            ot = op.tile([128, EB[0][1]], F32, tag="ot")
                            nc.vector.tensor_tensor(out=ot[:, :en], in0=ps[:, :en],
                                                    in1=x2[:, tt, eo:eo + en], op=ALU.add)
                            nc.sync.dma_start(
                                out=out.ap()[tt * 128:(tt + 1) * 128, eo:eo + en],
                                in_=ot[:, :en])


# ---------------- host side ----------------

def _tile_lhs(w, bf=True):
    """[K, M] -> [MT, 128, KT, 128] (per-m-tile contiguous lhsT blocks)."""
    K, M = w.shape
    t = w.reshape(K // 128, 128, M // 128, 128).transpose(2, 1, 0, 3)
    t = np.ascontiguousarray(t)
    return t.astype(ml_dtypes.bfloat16) if bf else t


def prepare_core_inputs(inputs, cfg: Cfg, b, r):
    E, H, T, R = cfg.E, cfg.H, cfg.T, cfg.R
    EBn = len(_blocks(E))
    x = np.asarray(inputs["x"])
    own_off = r * R
    im = {
        "x_b": np.ascontiguousarray(x[b]),
        "x_own": np.ascontiguousarray(x[b, own_off:own_off + R]),
        "ln1_w": np.asarray(inputs["ln1_w"]), "ln1_b": np.asarray(inputs["ln1_b"]),
        "ln2_w": np.asarray(inputs["ln2_w"]), "ln2_b": np.asarray(inputs["ln2_b"]),
        "bq": np.asarray(inputs["bq"]), "bk": np.asarray(inputs["bk"]),
        "bv": np.asarray(inputs["bv"]),
        "b1": np.asarray(inputs["b1"]), "b2a": np.asarray(inputs["b2a"]),
        "b2b": np.asarray(inputs["b2b"]), "b3": np.asarray(inputs["b3"]),
    }
    i_idx = own_off + np.arange(R)
    j_idx = np.arange(T)
    im["mask"] = np.where(j_idx[None, :] <= i_idx[:, None], 0.0,
                          -1e30).astype(np.float32)
    return im


def prepare_shared_weights(inputs, cfg: Cfg):
    E, H = cfg.E, cfg.H
    w3 = np.asarray(inputs["W3"])
    eb = _blocks(E)
    w3t = np.ascontiguousarray(
        w3.reshape(H // 128, 128, len(eb), eb[0][1]).transpose(2, 0, 1, 3)
    ).astype(ml_dtypes.bfloat16)
    return {
        "wqt": _tile_lhs(np.asarray(inputs["Wq"])),
        "wkt": _tile_lhs(np.asarray(inputs["Wk"])),
        "wv": np.asarray(inputs["Wv"]).astype(ml_dtypes.bfloat16),
        "w1t": _tile_lhs(np.asarray(inputs["W1"])),
        "w2at": _tile_lhs(np.asarray(inputs["W2a"])),
        "w2bt": _tile_lhs(np.asarray(inputs["W2b"])),
        "w3t": w3t,
    }


_PROGRAM_CACHE = {}


def get_program(cfg: Cfg):
    key = (cfg.E, cfg.H, cfg.T, cfg.R)
    if key not in _PROGRAM_CACHE:
        _PROGRAM_CACHE[key] = build_program(cfg)
    return _PROGRAM_CACHE[key]


def run(inputs, cfg: Cfg, trace=False):
    nc = get_program(cfg)
    shared = prepare_shared_weights(inputs, cfg)
    in_maps = []
    for c in range(N_CORES):
        b, r = c // 2, c % 2
        im = prepare_core_inputs(inputs, cfg, b, r)
        im.update(shared)
        in_maps.append(im)
    res = run_bass_kernel_spmd(nc, in_maps, core_ids=list(range(N_CORES)),
                               trace=trace)
    B = np.asarray(inputs["x"]).shape[0]
    T_full = np.asarray(inputs["x"]).shape[1]
    outp = np.empty((B, T_full, cfg.E), np.float32)
    for c in range(N_CORES):
        b, r = c // 2, c % 2
        outp[b, r * cfg.R:(r + 1) * cfg.R] = res.results[c]["out"]
    return outp, res


def _build_sharded_exec(nc, in_maps):
    """Mirror bass2jax.run_bass_via_pjrt but return a reusable timed runner."""
    import jax
    from jax.sharding import Mesh, PartitionSpec, NamedSharding
    from jax.experimental.shard_map import shard_map
    import concourse.mybir as mb
    from concourse import bass2jax

    bass2jax.install_neuronx_cc_hook()
    n_cores = len(in_maps)
    partition_name = (nc.partition_id_tensor.name
                      if nc.partition_id_tensor is not None else None)
    in_names, out_names, out_avals, zero_outs = [], [], [], []
    for alloc in nc.m.functions[0].allocations:
        if not isinstance(alloc, mb.MemoryLocationSet):
            continue
        name = alloc.memorylocations[0].name
        if alloc.kind == "ExternalInput":
            if name != partition_name:
                in_names.append(name)
        elif alloc.kind == "ExternalOutput":
            out_names.append(name)
            shape = tuple(alloc.tensor_shape)
            dtype = mb.dt.np(alloc.dtype)
            out_avals.append(jax.core.ShapedArray(shape, dtype))
            zero_outs.append(np.zeros(shape, dtype))
    n_params = len(in_names)
    n_outs = len(out_avals)
    all_names = in_names + out_names
    if partition_name is not None:
        all_names = all_names + [partition_name]

    def _body(*args):
        operands = list(args)
        if partition_name is not None:
            operands.append(bass2jax.partition_id_tensor())
        outs = bass2jax._bass_exec_p.bind(
            *operands,
            out_avals=tuple(out_avals),
            in_names=tuple(all_names),
            out_names=tuple(out_names),
            lowering_input_output_aliases=(),
            sim_require_finite=True,
            sim_require_nnan=True,
            nc=nc,
        )
        return tuple(outs)

    devices = jax.devices()[:n_cores]
    mesh = Mesh(np.asarray(devices), ("core",))
    in_specs = (PartitionSpec("core"),) * (n_params + n_outs)
    out_specs = (PartitionSpec("core"),) * n_outs
    donate = tuple(range(n_params, n_params + n_outs))
    sharded = jax.jit(
        shard_map(_body, mesh=mesh, in_specs=in_specs, out_specs=out_specs,
                  check_rep=False),
        donate_argnums=donate, keep_unused=True)

    sh = NamedSharding(mesh, PartitionSpec("core"))
    concat_in = [
        jax.device_put(
            np.concatenate([np.asarray(in_maps[c][nm]) for c in range(n_cores)],
                           axis=0), sh)
        for nm in in_names
    ]

    def make_zeros():
        return [jax.device_put(
            np.zeros((n_cores * z.shape[0], *z.shape[1:]), z.dtype), sh)
            for z in zero_outs]

    def runner():
        zs = make_zeros()
        for z in zs:
            z.block_until_ready()
        t0 = time.perf_counter()
        outs = sharded(*concat_in, *zs)
        for o in outs:
            o.block_until_ready()
        return time.perf_counter() - t0, outs

    return runner, out_names


def time_exec(inputs, cfg: Cfg, iters=5):
    nc = get_program(cfg)
    shared = prepare_shared_weights(inputs, cfg)
    in_maps = []
    for c in range(N_CORES):
        b, r = c // 2, c % 2
        im = prepare_core_inputs(inputs, cfg, b, r)
        im.update(shared)
        in_maps.append(im)
    runner, _ = _build_sharded_exec(nc, in_maps)
    times = []
    for _ in range(iters):
        dt, _ = runner()
        times.append(dt)
    return times


def time_trivial(iters=5):
    """Dispatch-overhead baseline: near-empty SPMD kernel, same exec path."""
    nc = bacc.Bacc("TRN2", target_bir_lowering=False, debug=False,
                   num_devices=N_CORES)
    xi = nc.dram_tensor("xi", [128, 128], F32, kind="ExternalInput")
    yo = nc.dram_tensor("yo", [128, 128], F32, kind="ExternalOutput")
    with tile.TileContext(nc) as tc:
        with tc.tile_pool(name="p", bufs=1) as pool:
            t = pool.tile([128, 128], F32)
            nc.sync.dma_start(out=t[:], in_=xi.ap())
            nc.sync.dma_start(out=yo.ap(), in_=t[:])
    nc.compile()
    in_maps = [{"xi": np.zeros((128, 128), np.float32)} for _ in range(N_CORES)]
    runner, _ = _build_sharded_exec(nc, in_maps)
    times = []
    for _ in range(iters):
        dt, _ = runner()
        times.append(dt)
    return times


def kernel(**inputs) -> np.ndarray:
    cfg = Cfg(E=1024, H=4096, T=2048, R=1024)
    outp, _ = run(inputs, cfg)
    return outp
